# revision 8
# baseline (speedup 1.0000x reference)
"""Trainium2 Bass kernel for a 3-layer BERT-style cross-attention encoder.

Problem: s1 [64,256,768] attends to s2 [64,197,768], 3 layers, H=768, 12 heads,
FF=3072, exact-erf gelu, LN eps=1e-12.

Strategy: data-parallel over batch across 8 NeuronCores (8 batch elems/core),
no collectives. All matmuls run in float32r (tf32-class, full PE rate for
free-dim >= 256) accumulating fp32 in PSUM.

Layouts (per core, B_loc=8, T1=2048 s1-tokens, T2=1576 s2-tokens):
  - "transposed" activations X^T as [128, C, tokens] where feature
    f = c*128 + p (p = partition). Matmul-ready: contraction on partitions.
  - "natural" activations as [128, ntiles, 768] where token t = i*128 + p.
    LayerNorm/softmax-friendly (free-dim reductions).
Weights are host-pre-tiled so every DMA is contiguous. Large inter-phase
activations (s1T, s1n, ctxT, attn_out natural+transposed) round-trip through
DRAM scratch so SBUF holds only each phase's working set (pools are a LIFO
stack; a pool's footprint is the sum of its tags for its whole lifetime).
"""

import os
import sys
from contextlib import ExitStack

for _p in ("/opt/trn_rl_repo", "/root/.axon_site/_ro/trn_rl_repo"):
    if os.path.isdir(_p) and _p not in sys.path:
        sys.path.insert(0, _p)

import numpy as np

import concourse.bass as bass
import concourse.tile as tile
from concourse import bacc, mybir
from concourse.bass_utils import run_bass_kernel_spmd
from concourse.masks import make_identity

F32 = mybir.dt.float32
F32R = mybir.dt.float32r
AF = mybir.ActivationFunctionType
ALU = mybir.AluOpType

N_CORES = 8
N_LAYERS = 3
H = 768
C = H // 128          # 6 hidden chunks
NH = 12
HD = 64
FF = 3072
CF = FF // 128        # 24 ff chunks
L1 = 256
L2 = 197
B_LOC = 8             # batch elems per core
T1 = B_LOC * L1       # 2048
T2 = B_LOC * L2       # 1576
QT1 = T1 // 128       # 16 q-tiles of 128
TT1 = T1 // 512       # 4 token tiles of 512
KC_N = (128, L2 - 128)  # k chunks per batch elem: 128 + 69
EPS = 1e-12
SM_SCALE = 0.125      # 1/sqrt(64)


def build_module(g):
    """g: dict of gates (use_mask, use_bq, ..., use_ln1, use_ln2)."""
    nc = bacc.Bacc(None)

    def P(name, shape, dt, out=False):
        return nc.declare_dram_parameter(name, list(shape), dt, isOutput=out)

    s1T_in = P("s1T", [128, C, T1], F32R)
    s1n_in = P("s1n", [128, QT1, H], F32)
    s2T_in = P("s2T", [128, C, T2], F32R)
    maskT_in = P("maskT", [128, 2, B_LOC], F32) if g["use_mask"] else None
    wq_in = P("wq", [N_LAYERS, 128, C, H], F32R)
    wk_in = P("wk", [N_LAYERS, 128, C, H], F32R)
    wv_in = P("wv", [N_LAYERS, 128, C, H], F32R)
    wo_in = P("wo", [N_LAYERS, 128, C, H], F32R)
    wi_in = P("wi", [N_LAYERS, 128, C, FF], F32R)
    wf_in = P("wf", [N_LAYERS, CF, 128, H], F32R)
    bq_in = P("bq", [N_LAYERS, 128, C], F32) if g["use_bq"] else None
    bk_in = P("bk", [N_LAYERS, 128, C], F32) if g["use_bk"] else None
    bi_in = P("bi", [N_LAYERS, 128, CF], F32) if g["use_bi"] else None
    rows_in = P("rows", [N_LAYERS, 7, H], F32)  # bv,bo,bf,ln1w,ln1b,ln2w,ln2b
    ones_in = P("ones", [128, NH, 1], F32R)
    out_d = P("out", [128, QT1, H], F32, out=True)

    with tile.TileContext(nc) as tc, ExitStack() as ctx:
        base = ctx.enter_context(tc.tile_pool(name="base", bufs=1))
        rows = ctx.enter_context(tc.tile_pool(name="rows", bufs=2))
        dram = ctx.enter_context(tc.tile_pool(name="dram", bufs=1, space="DRAM"))

        s1T_scr = dram.tile([128, C, T1], F32R, tag="s1T_scr")
        s1n_scr = dram.tile([128, QT1, H], F32, tag="s1n_scr")
        anat_scr = dram.tile([128, QT1, H], F32, tag="anat_scr")
        aT_scr = dram.tile([128, C, T1], F32R, tag="aT_scr")
        ctxT_scr = dram.tile([128, C, T1], F32R, tag="ctxT_scr")

        ident = base.tile([128, 128], F32)
        make_identity(nc, ident)
        eps_t = base.tile([128, 1], F32)
        nc.vector.memset(eps_t, EPS)
        if g["use_mask"]:
            maskT_sb = base.tile([128, 2, B_LOC], F32)
            nc.sync.dma_start(out=maskT_sb[:], in_=maskT_in[:])

        def row_bcast(src_row, name):
            t = rows.tile([128, H], F32, tag=name, name=name + "_t")
            nc.sync.dma_start(out=t[:], in_=src_row.to_broadcast((128, H)))
            return t

        def ln_epilogue(x, use_g, w_bc, b_bc, uid):
            """LayerNorm over free dim (768) of natural f32 tile x [128, 768]."""
            stats = base.tile([128, 3, 6], F32, tag="lnstats", bufs=2,
                              name=f"st_{uid}")
            for i in range(3):
                nc.vector.bn_stats(out=stats[:, i, :],
                                   in_=x[:, i * 256:(i + 1) * 256])
            mv = base.tile([128, 2], F32, tag="lnmv", bufs=2, name=f"mv_{uid}")
            nc.vector.bn_aggr(out=mv[:], in_=stats[:])
            sd = base.tile([128, 1], F32, tag="lnsd", bufs=2, name=f"sd_{uid}")
            nc.scalar.activation(out=sd[:], in_=mv[:, 1:2], func=AF.Sqrt,
                                 bias=eps_t[:], scale=1.0)
            nc.vector.reciprocal(out=sd[:], in_=sd[:])
            y = base.tile([128, H], F32, tag="lnout", bufs=2, name=f"y_{uid}")
            nc.vector.tensor_scalar(
                out=y[:], in0=x[:], scalar1=mv[:, 0:1], scalar2=sd[:],
                op0=ALU.subtract, op1=ALU.mult)
            if use_g:
                nc.vector.tensor_mul(out=y[:], in0=y[:], in1=w_bc[:])
                nc.vector.tensor_add(out=y[:], in0=y[:], in1=b_bc[:])
            return y

        def transpose_out(psp, y, dst, q, uid):
            """PE-transpose natural tile y [128, 768] into dst[:, :, q*128±128]."""
            stg = base.tile([128, C, 128], F32R, tag="tstage", bufs=2,
                            name=f"stg_{uid}")
            for ch in range(C):
                tp = psp.tile([128, 128], F32, tag="t_ps", bufs=2,
                              name=f"tp_{uid}_{ch}")
                nc.tensor.transpose(tp[:], y[:, ch * 128:(ch + 1) * 128], ident[:])
                nc.vector.tensor_copy(out=stg[:, ch, :], in_=tp[:])
            nc.sync.dma_start(out=dst[:, :, q * 128:(q + 1) * 128], in_=stg[:])

        for l in range(N_LAYERS):
            s1T_src = s1T_in if l == 0 else s1T_scr
            s1n_src = s1n_in if l == 0 else s1n_scr
            last = l == N_LAYERS - 1
            s1n_dst = out_d if last else s1n_scr

            # ---- per-layer constants (gated; absent in the default setup) --
            bv_bc = row_bcast(rows_in[l, 0], "bv") if g["use_bv"] else None
            bo_bc = row_bcast(rows_in[l, 1], "bo") if g["use_bo"] else None
            bf_bc = row_bcast(rows_in[l, 2], "bf") if g["use_bf"] else None
            ln1w_bc = row_bcast(rows_in[l, 3], "ln1w") if g["use_ln1"] else None
            ln1b_bc = row_bcast(rows_in[l, 4], "ln1b") if g["use_ln1"] else None
            ln2w_bc = row_bcast(rows_in[l, 5], "ln2w") if g["use_ln2"] else None
            ln2b_bc = row_bcast(rows_in[l, 6], "ln2b") if g["use_ln2"] else None
            bq_sb = bk_sb = bi_sb = None
            if g["use_bq"]:
                bq_sb = rows.tile([128, C], F32, tag="bq", name=f"bq{l}")
                nc.sync.dma_start(out=bq_sb[:], in_=bq_in[l])
            if g["use_bk"]:
                bk_sb = rows.tile([128, C], F32, tag="bk", name=f"bk{l}")
                nc.sync.dma_start(out=bk_sb[:], in_=bk_in[l])
            if g["use_bi"]:
                bi_sb = rows.tile([128, CF], F32, tag="bi", name=f"bi{l}")
                nc.sync.dma_start(out=bi_sb[:], in_=bi_in[l])

            with tc.tile_pool(name=f"pw{l}", bufs=2) as pw:
                with tc.tile_pool(name=f"pa2_{l}", bufs=1) as pa2:
                    QT = pa2.tile([128, C, T1], F32R, tag="QT", name=f"QT{l}")
                    KT = pa2.tile([128, C, T2], F32R, tag="KT", name=f"KT{l}")

                    with tc.tile_pool(name=f"pa1_{l}", bufs=1) as pa1, \
                         tc.tile_pool(name=f"psA{l}", bufs=1,
                                      space="PSUM") as psA:
                        # ===== A1: Q^T = proj(Wq; s1T) =====
                        s1T_t = pa1.tile([128, C, T1], F32R, tag="actT",
                                         name=f"s1T_t{l}")
                        nc.sync.dma_start(out=s1T_t[:], in_=s1T_src[:])
                        wq_sb = pw.tile([128, C, H], F32R, tag="wqkvo",
                                        name=f"wq{l}")
                        nc.sync.dma_start(out=wq_sb[:], in_=wq_in[l])
                        for feat in range(C):
                            for t in range(TT1):
                                pq = psA.tile([128, 512], F32, tag="proj_ps",
                                             bufs=4, name=f"pq{l}_{feat}_{t}")
                                for ch in range(C):
                                    nc.tensor.matmul(
                                        pq[:],
                                        wq_sb[:, ch, feat * 128:(feat + 1) * 128],
                                        s1T_t[:, ch, t * 512:(t + 1) * 512],
                                        start=(ch == 0), stop=(ch == C - 1))
                                dst = QT[:, feat, t * 512:(t + 1) * 512]
                                if g["use_bq"]:
                                    nc.scalar.activation(
                                        out=dst, in_=pq[:], func=AF.Copy,
                                        bias=bq_sb[:, feat:feat + 1])
                                else:
                                    nc.scalar.activation(out=dst, in_=pq[:],
                                                         func=AF.Copy)

                        # ===== A2: K^T = proj(Wk; s2T) =====
                        s2T_t = pa1.tile([128, C, T2], F32R, tag="actT",
                                         name=f"s2T_t{l}")
                        nc.sync.dma_start(out=s2T_t[:], in_=s2T_in[:])
                        wk_sb = pw.tile([128, C, H], F32R, tag="wqkvo",
                                        name=f"wk{l}")
                        nc.sync.dma_start(out=wk_sb[:], in_=wk_in[l])
                        for feat in range(C):
                            for t in range(4):
                                nsz = min(512, T2 - t * 512)
                                pk = psA.tile([128, 512], F32, tag="proj_ps",
                                             bufs=4, name=f"pk{l}_{feat}_{t}")
                                for ch in range(C):
                                    nc.tensor.matmul(
                                        pk[:, :nsz],
                                        wk_sb[:, ch, feat * 128:(feat + 1) * 128],
                                        s2T_t[:, ch, t * 512:t * 512 + nsz],
                                        start=(ch == 0), stop=(ch == C - 1))
                                dst = KT[:, feat, t * 512:t * 512 + nsz]
                                if g["use_bk"]:
                                    nc.scalar.activation(
                                        out=dst, in_=pk[:, :nsz], func=AF.Copy,
                                        bias=bk_sb[:, feat:feat + 1])
                                else:
                                    nc.scalar.activation(out=dst, in_=pk[:, :nsz],
                                                         func=AF.Copy)

                    # ===== A3: per-batch V proj + attention =====
                    wv_sb = pw.tile([128, C, H], F32R, tag="wqkvo", name=f"wv{l}")
                    nc.sync.dma_start(out=wv_sb[:], in_=wv_in[l])
                    with tc.tile_pool(name=f"patt{l}", bufs=1) as patt, \
                         tc.tile_pool(name=f"psT{l}", bufs=1,
                                      space="PSUM") as psT:
                        for b in range(B_LOC):
                            s2b = patt.tile([128, C, L2], F32R, tag="s2b", bufs=2,
                                            name=f"s2b{l}_{b}")
                            nc.sync.dma_start(
                                out=s2b[:], in_=s2T_in[:, :, b * L2:(b + 1) * L2])
                            # V natural [k, h, 65]; col 64 = ones (sumexp trick)
                            V_sb = patt.tile([128, 2, NH, HD + 1], F32R, tag="V",
                                             bufs=2, name=f"V{l}_{b}")
                            for kc, kn in enumerate(KC_N):
                                vp = psT.tile([128, H], F32, tag="v_ps",
                                             name=f"vp{l}_{b}_{kc}")
                                for ch in range(C):
                                    nc.tensor.matmul(
                                        vp[:kn, 0:512],
                                        s2b[:, ch, kc * 128:kc * 128 + kn],
                                        wv_sb[:, ch, 0:512],
                                        start=(ch == 0), stop=(ch == C - 1))
                                    nc.tensor.matmul(
                                        vp[:kn, 512:H],
                                        s2b[:, ch, kc * 128:kc * 128 + kn],
                                        wv_sb[:, ch, 512:H],
                                        start=(ch == 0), stop=(ch == C - 1))
                                vsrc = vp[:kn, :].rearrange("p (h d) -> p h d",
                                                            h=NH)
                                if g["use_bv"]:
                                    nc.vector.tensor_add(
                                        out=V_sb[:kn, kc, :, 0:HD], in0=vsrc,
                                        in1=bv_bc[:kn, :].rearrange(
                                            "p (h d) -> p h d", h=NH))
                                else:
                                    nc.vector.tensor_copy(
                                        out=V_sb[:kn, kc, :, 0:HD], in_=vsrc)
                                nc.sync.dma_start(
                                    out=V_sb[:kn, kc, :, HD:HD + 1],
                                    in_=ones_in[:kn])

                            for half in range(2):  # heads 6*half .. 6*half+5
                                # cb: [d(0:64)+sumexp(64), parity, chunk, q]
                                cb = psT.tile([128, 2, 3, L1], F32, tag="ctx_big",
                                             name=f"cb{l}_{b}_{half}")
                                for j in range(6):
                                    h = 6 * half + j
                                    p0 = 64 * (h % 2)
                                    chh = h // 2
                                    sp = psT.tile([128, 2, L1], F32, tag="sc_ps",
                                                 bufs=2, name=f"sp{l}_{b}_{h}")
                                    expT = patt.tile([128, 2, L1], F32R,
                                                     tag="expT", bufs=3,
                                                     name=f"e{l}_{b}_{h}")
                                    for kc, kn in enumerate(KC_N):
                                        k0 = b * L2 + kc * 128
                                        nc.tensor.matmul(
                                            sp[:kn, kc, :],
                                            KT[p0:p0 + 64, chh, k0:k0 + kn],
                                            QT[p0:p0 + 64, chh,
                                               b * L1:(b + 1) * L1],
                                            start=True, stop=True)
                                        if g["use_mask"]:
                                            nc.scalar.activation(
                                                out=expT[:kn, kc, :],
                                                in_=sp[:kn, kc, :], func=AF.Exp,
                                                bias=maskT_sb[:kn, kc, b:b + 1],
                                                scale=SM_SCALE)
                                        else:
                                            nc.scalar.activation(
                                                out=expT[:kn, kc, :],
                                                in_=sp[:kn, kc, :], func=AF.Exp,
                                                scale=SM_SCALE)
                                    for kc, kn in enumerate(KC_N):
                                        nc.tensor.matmul(
                                            cb[0:HD + 1, j % 2, j // 2, :],
                                            V_sb[:kn, kc, h, :], expT[:kn, kc, :],
                                            start=(kc == 0), stop=(kc == 1))
                                # row 64 of each head block = sumexp -> normalize
                                rrow = patt.tile([1, 2, 3, L1], F32, tag="rrow",
                                                 bufs=2, name=f"rr{l}_{b}_{half}")
                                nc.vector.reciprocal(out=rrow[:],
                                                     in_=cb[HD:HD + 1, :, :, :])
                                rbc = patt.tile([64, 2, 3, L1], F32, tag="rbc",
                                                bufs=2, name=f"rb{l}_{b}_{half}")
                                nc.gpsimd.partition_broadcast(
                                    rbc[:].rearrange("p a b q -> p (a b q)"),
                                    rrow[:].rearrange("o a b q -> o (a b q)"))
                                ce = patt.tile([64, 3, L1], F32R, tag="ctmp",
                                               bufs=3, name=f"ce{l}_{b}_{half}")
                                co = patt.tile([64, 3, L1], F32R, tag="ctmp",
                                               bufs=3, name=f"co{l}_{b}_{half}")
                                nc.vector.tensor_mul(
                                    out=ce[:], in0=cb[0:64, 0, :, :],
                                    in1=rbc[:, 0, :, :])
                                nc.vector.tensor_mul(
                                    out=co[:], in0=cb[0:64, 1, :, :],
                                    in1=rbc[:, 1, :, :])
                                c0 = 3 * half
                                nc.sync.dma_start(
                                    out=ctxT_scr[0:64, c0:c0 + 3,
                                                 b * L1:(b + 1) * L1], in_=ce[:])
                                nc.sync.dma_start(
                                    out=ctxT_scr[64:128, c0:c0 + 3,
                                                 b * L1:(b + 1) * L1], in_=co[:])

                # ===== C: o-proj + residual + LN1 =====
                wo_sb = pw.tile([128, C, H], F32R, tag="wqkvo", name=f"wo{l}")
                nc.sync.dma_start(out=wo_sb[:], in_=wo_in[l])
                with tc.tile_pool(name=f"pcc{l}", bufs=1) as pcc, \
                     tc.tile_pool(name=f"psC{l}", bufs=1, space="PSUM") as psC:
                    for q in range(QT1):
                        cq = pcc.tile([128, C, 128], F32R, tag="ctxq", bufs=3,
                                      name=f"cq{l}_{q}")
                        nc.sync.dma_start(
                            out=cq[:], in_=ctxT_scr[:, :, q * 128:(q + 1) * 128])
                        po = psC.tile([128, H], F32, tag="o_ps", bufs=2,
                                     name=f"po{l}_{q}")
                        for ch in range(C):
                            nc.tensor.matmul(
                                po[:, 0:512], cq[:, ch, :], wo_sb[:, ch, 0:512],
                                start=(ch == 0), stop=(ch == C - 1))
                            nc.tensor.matmul(
                                po[:, 512:H], cq[:, ch, :], wo_sb[:, ch, 512:H],
                                start=(ch == 0), stop=(ch == C - 1))
                        s1n_t = base.tile([128, H], F32, tag="res_t", bufs=2,
                                          name=f"s1n_t{l}_{q}")
                        nc.sync.dma_start(out=s1n_t[:], in_=s1n_src[:, q, :])
                        x = base.tile([128, H], F32, tag="lnx", bufs=2,
                                      name=f"x{l}_{q}")
                        nc.vector.tensor_add(out=x[:], in0=po[:], in1=s1n_t[:])
                        if g["use_bo"]:
                            nc.vector.tensor_add(out=x[:], in0=x[:], in1=bo_bc[:])
                        y = ln_epilogue(x, g["use_ln1"], ln1w_bc, ln1b_bc,
                                        f"c{l}_{q}")
                        nc.sync.dma_start(out=anat_scr[:, q, :], in_=y[:])
                        transpose_out(psC, y, aT_scr, q, f"c{l}_{q}")

            # ===== D: FFN per 512-token tile =====
            with tc.tile_pool(name=f"pd{l}", bufs=1) as pd, \
                 tc.tile_pool(name=f"psD{l}", bufs=1, space="PSUM") as psD:
                wi_sb = pd.tile([128, C, FF], F32R, tag="wi_sb", name=f"wi{l}")
                nc.sync.dma_start(out=wi_sb[:], in_=wi_in[l])
                for t in range(TT1):
                    aT_t = pd.tile([128, C, 512], F32R, tag="aT_t", bufs=2,
                                   name=f"aT_t{l}_{t}")
                    nc.sync.dma_start(out=aT_t[:],
                                      in_=aT_scr[:, :, t * 512:(t + 1) * 512])
                    interT = pd.tile([128, CF, 512], F32R, tag="interT",
                                     name=f"interT{l}_{t}")
                    for ff in range(CF):
                        pf = psD.tile([128, 512], F32, tag="f_ps", bufs=2,
                                     name=f"pf{l}_{t}_{ff}")
                        for ch in range(C):
                            nc.tensor.matmul(
                                pf[:], wi_sb[:, ch, ff * 128:(ff + 1) * 128],
                                aT_t[:, ch, :], start=(ch == 0),
                                stop=(ch == C - 1))
                        if g["use_bi"]:
                            nc.scalar.activation(
                                out=interT[:, ff, :], in_=pf[:], func=AF.Gelu,
                                bias=bi_sb[:, ff:ff + 1])
                        else:
                            nc.scalar.activation(
                                out=interT[:, ff, :], in_=pf[:], func=AF.Gelu)
                    # FFN2: natural out [q,768]; contract 3072; wf in 384-halves
                    po2 = [psD.tile([128, 384], F32, tag=f"o2_{qq}",
                                   name=f"po2_{l}_{t}_{qq}") for qq in range(4)]
                    o2 = [pd.tile([128, H], F32, tag=f"o2sb_{qq}",
                                  name=f"o2_{l}_{t}_{qq}") for qq in range(4)]
                    for hf in range(2):
                        for cf in range(CF):
                            wf_t = pd.tile([128, 384], F32R, tag="wf_t", bufs=3,
                                           name=f"wf{l}_{t}_{hf}_{cf}")
                            nc.sync.dma_start(
                                out=wf_t[:],
                                in_=wf_in[l, cf, :, hf * 384:(hf + 1) * 384])
                            for qq in range(4):
                                nc.tensor.matmul(
                                    po2[qq][:],
                                    interT[:, cf, qq * 128:(qq + 1) * 128],
                                    wf_t[:], start=(cf == 0),
                                    stop=(cf == CF - 1))
                        for qq in range(4):
                            nc.scalar.activation(
                                out=o2[qq][:, hf * 384:(hf + 1) * 384],
                                in_=po2[qq][:], func=AF.Copy)
                    for qq in range(4):
                        q = t * 4 + qq
                        anat_t = base.tile([128, H], F32, tag="res_t", bufs=2,
                                           name=f"an{l}_{t}_{qq}")
                        nc.sync.dma_start(out=anat_t[:], in_=anat_scr[:, q, :])
                        x2 = base.tile([128, H], F32, tag="lnx", bufs=2,
                                       name=f"x2{l}_{t}_{qq}")
                        nc.vector.tensor_add(out=x2[:], in0=o2[qq][:],
                                             in1=anat_t[:])
                        if g["use_bf"]:
                            nc.vector.tensor_add(out=x2[:], in0=x2[:],
                                                 in1=bf_bc[:])
                        y2 = ln_epilogue(x2, g["use_ln2"], ln2w_bc, ln2b_bc,
                                         f"d{l}_{t}_{qq}")
                        nc.sync.dma_start(out=s1n_dst[:, q, :], in_=y2[:])
                        if not last:
                            transpose_out(psD, y2, s1T_scr, q, f"d{l}_{t}_{qq}")

    nc.finalize()
    return nc


# ---------------------------------------------------------------------------
# Host-side tiling
# ---------------------------------------------------------------------------

def _tile_T(x2d):  # [T, H] -> [128, C, T] transposed layout
    T = x2d.shape[0]
    return np.ascontiguousarray(x2d.T.reshape(C, 128, T).transpose(1, 0, 2))


def _tile_nat(x2d):  # [T, H] -> [128, T//128, H] natural layout
    T = x2d.shape[0]
    return np.ascontiguousarray(x2d.reshape(T // 128, 128, H).transpose(1, 0, 2))


def _prep_shared(inputs):
    """Weight tensors, tiled once (shared by all cores)."""
    def tl(W):  # [L, 768, X] -> [L, 128, 6, X]
        L, K, X = W.shape
        return np.ascontiguousarray(W.reshape(L, C, 128, X).transpose(0, 2, 1, 3))

    Wq, Wk, Wv, Wo, Wi = (np.asarray(inputs[k], np.float32)
                          for k in ("Wq", "Wk", "Wv", "Wo", "Wi"))
    Wf = np.asarray(inputs["Wf"], np.float32)
    rows = np.stack([np.asarray(inputs[k], np.float32) for k in
                     ("bv", "bo", "bf", "ln1_w", "ln1_b", "ln2_w", "ln2_b")],
                    axis=1)  # [3, 7, 768]

    def bcol(b, n):  # [L, n*128] -> [L, 128, n]
        return np.ascontiguousarray(
            np.asarray(b, np.float32).reshape(N_LAYERS, n, 128).transpose(0, 2, 1))

    return {
        "wq": tl(Wq), "wk": tl(Wk), "wv": tl(Wv), "wo": tl(Wo), "wi": tl(Wi),
        "wf": np.ascontiguousarray(Wf.reshape(N_LAYERS, CF, 128, H)),
        "rows": np.ascontiguousarray(rows),
        "bq": bcol(inputs["bq"], C), "bk": bcol(inputs["bk"], C),
        "bi": bcol(inputs["bi"], CF),
        "ones": np.ones((128, NH, 1), np.float32),
    }


def make_gates(inputs):
    return {
        "use_mask": bool(np.any(np.asarray(inputs["s2_attention_mask"]) != 0)),
        "use_bq": bool(np.any(np.asarray(inputs["bq"]) != 0)),
        "use_bk": bool(np.any(np.asarray(inputs["bk"]) != 0)),
        "use_bv": bool(np.any(np.asarray(inputs["bv"]) != 0)),
        "use_bo": bool(np.any(np.asarray(inputs["bo"]) != 0)),
        "use_bi": bool(np.any(np.asarray(inputs["bi"]) != 0)),
        "use_bf": bool(np.any(np.asarray(inputs["bf"]) != 0)),
        "use_ln1": not (np.all(np.asarray(inputs["ln1_w"]) == 1)
                        and np.all(np.asarray(inputs["ln1_b"]) == 0)),
        "use_ln2": not (np.all(np.asarray(inputs["ln2_w"]) == 1)
                        and np.all(np.asarray(inputs["ln2_b"]) == 0)),
    }


def make_in_maps(inputs):
    s1 = np.asarray(inputs["s1_hidden_states"], np.float32)
    s2 = np.asarray(inputs["s2_hidden_states"], np.float32)
    mask = np.asarray(inputs["s2_attention_mask"], np.float32)
    shared = _prep_shared(inputs)
    in_maps = []
    for c in range(N_CORES):
        s1c = s1[c * B_LOC:(c + 1) * B_LOC].reshape(T1, H)
        s2c = s2[c * B_LOC:(c + 1) * B_LOC].reshape(T2, H)
        mc = mask[c * B_LOC:(c + 1) * B_LOC, 0, 0, :]  # [8, 197]
        maskT = np.zeros((128, 2, B_LOC), np.float32)
        maskT[:, 0, :] = mc[:, :128].T
        maskT[:L2 - 128, 1, :] = mc[:, 128:].T
        in_maps.append({
            "s1T": _tile_T(s1c), "s1n": _tile_nat(s1c), "s2T": _tile_T(s2c),
            "maskT": maskT, **shared,
        })
    return in_maps


def assemble_out(results):
    out = np.empty((N_CORES * B_LOC, L1, H), np.float32)
    for c in range(N_CORES):
        o = results[c]["out"]  # [128, 16, 768]
        out[c * B_LOC:(c + 1) * B_LOC] = o.transpose(1, 0, 2).reshape(B_LOC, L1, H)
    return out


def kernel(**inputs):
    assert inputs["s1_hidden_states"].shape[0] == N_CORES * B_LOC
    nc = build_module(make_gates(inputs))
    in_maps = make_in_maps(inputs)
    res = run_bass_kernel_spmd(nc, in_maps, core_ids=list(range(N_CORES)))
    return assemble_out(res.results)
